# revision 26
# baseline (speedup 1.0000x reference)
"""Trainium2 Bass kernel for the CA2 dense-transformer problem.

Math (per batch b of 8, S=2048, D=512):
    Q1 = X @ W_xq.T + b_xq            # [S, D]
    Q2 = Y @ W_yq.T + b_yq
    Qc = concat(Q1, Q2, -1)           # [S, 2D]
    K  = Qc @ W_fk.T + b_fk
    V  = Qc @ W_fv.T + b_fv
    out = X + Y + softmax(Q1 K^T / sqrt(D)) V + softmax(Q2 K^T / sqrt(D)) V

Sharding: pure data-parallel over batch; core i handles batch i.

Numerics: every matmul runs in fp8e4 (e4m3) with DoubleRow perf mode
(measured ~0.5 PE cycles per output column on TRN2, 4x the fp32r rate),
accumulating in fp32 PSUM.  Weights are pre-scaled by 2^12 on the host so
their small uniform(-0.03..0.04) entries land in e4m3's normal range; the
2^-12 descale is folded into the fp32 epilogue.  The attention 1/sqrt(D)
scale is folded into the Exp activation's scale operand.  The softmax
denominator, residual X+Y, and output all stay fp32.  The attention
contribution is ~4% of the output norm, so fp8's ~2-3% elementwise error
dilutes to <1e-3 relative error.

Schedule: engines execute their queues in emission order.  The
projections are emitted pipelined per 512-token slice (Qx, Qy, K, V per
slice), and the first attention block (pass 1, q-block 0 — whose queries
only need slice 0) starts after just two slices, with the remaining two
slices interleaved between its key-chunks, sharing one 3-buffer PSUM
pool between projection psums and score psums (3 + 4 po + 1 pd = 8
banks).  Non-PE work is spread: scalar does exp + the K and half the Q
epilogues, DVE the rest + PSUM evacuations, GPSIMD residual init and the
final accumulate into racc.  The softmax denominator runs on the tensor
engine: a DoubleRow ones-matmul per key-chunk accumulates lane sums into
a [2, 512] PSUM strip, transposed to per-partition orientation by 4 tiny
PE transposes.
"""

import sys

if "/opt/trn_rl_repo" not in sys.path:
    sys.path.insert(0, "/opt/trn_rl_repo")

import ml_dtypes
import numpy as np

import concourse.bass as bass  # noqa: F401  (bass types used via tile/bacc)
import concourse.mybir as mybir
import concourse.tile as tile
from concourse import bacc
from concourse.bass_utils import run_bass_kernel_spmd

P = 128          # SBUF partitions
S = 2048         # tokens per batch
D = 512          # feature dim
NQT = S // P     # 16 token tiles
NET = D // P     # 4 feature tiles of D
NCT = 2 * D // P # 8 feature tiles of 2D
NE2 = NET // 2   # 2 double (256-deep) feature tiles of D
NC2 = NCT // 2   # 4 double feature tiles of 2D
NK2 = NQT // 2   # 8 double key tiles
NSS = S // 512   # 4 512-wide token column slices
QB = 512         # q-block columns processed together in attention
NQB = S // QB    # 4
NQS = QB // P    # 4 q-subtiles per block
FP = mybir.dt.float32
F8 = mybir.dt.float8e4
DR = mybir.MatmulPerfMode.DoubleRow
WS = 2.0 ** 12   # host-side weight pre-scale (max |w|*WS ~ 181 < 240)
IWS = 1.0 / WS

_CACHE = {}


def _build(reps: int = 1):
    nc = bacc.Bacc("TRN2", target_bir_lowering=False, debug=False)

    xt_d = nc.dram_tensor("xt", [NET, P, S], F8, kind="ExternalInput")
    yt_d = nc.dram_tensor("yt", [NET, P, S], F8, kind="ExternalInput")
    x_d = nc.dram_tensor("x", [NQT, P, D], FP, kind="ExternalInput")
    y_d = nc.dram_tensor("y", [NQT, P, D], FP, kind="ExternalInput")
    wxq_d = nc.dram_tensor("wxq", [NET, P, D], F8, kind="ExternalInput")
    wyq_d = nc.dram_tensor("wyq", [NET, P, D], F8, kind="ExternalInput")
    wfk_d = nc.dram_tensor("wfk", [NCT, P, D], F8, kind="ExternalInput")
    wfv_d = nc.dram_tensor("wfv", [NCT, P, D], F8, kind="ExternalInput")
    bq_d = nc.dram_tensor("bq", [P, 12], FP, kind="ExternalInput")
    bfv_d = nc.dram_tensor("bfv", [P, D], FP, kind="ExternalInput")
    sel_d = nc.dram_tensor("sel", [2, NQS, 2 * NQS], FP, kind="ExternalInput")
    out_d = nc.dram_tensor("out", [NQT, P, D], FP, kind="ExternalOutput")

    Exp = mybir.ActivationFunctionType.Exp
    Ident = mybir.ActivationFunctionType.Identity
    mult = mybir.AluOpType.mult
    add = mybir.AluOpType.add
    ATT_SCALE = float(1.0 / np.sqrt(np.float32(D)))

    with tile.TileContext(nc) as tc:
        for _rep in range(reps):
            with (
                tc.tile_pool(name="main", bufs=1) as main,
                tc.tile_pool(name="work", bufs=3) as work,
            ):
                q1t = main.tile([P, NET, S], F8, tag="q1t")
                q2t = main.tile([P, NET, S], F8, tag="q2t")
                kft = main.tile([P, NET, S], F8, tag="kft")
                vf = main.tile([P, NQT, D], F8, tag="vf")
                racc = main.tile([P, NQT, D], FP, tag="racc")
                bq = main.tile([P, 12], FP, tag="bq")
                bfv = main.tile([P, D], FP, tag="bfv")
                # DoubleRow ldweights requires the k-pair dim stride to be a
                # multiple of 16 elements, so pad the ones tile to [P, 2, 16].
                ones8 = main.tile([P, 2, 16], F8, tag="ones8")
                sel = main.tile([2, NQS, 2 * NQS], FP, tag="sel")
                nc.scalar.dma_start(bq[:], bq_d[:])
                nc.scalar.dma_start(bfv[:], bfv_d[:])
                nc.scalar.dma_start(sel[:], sel_d[:])
                nc.vector.memset(ones8[:], 1.0)

                with (
                    tc.tile_pool(name="stA", bufs=1) as stA,
                    tc.tile_pool(name="psS", bufs=3, space="PSUM") as psS,
                    tc.tile_pool(name="esp", bufs=3) as esp,
                    tc.tile_pool(name="rcp", bufs=2) as rcp,
                    tc.tile_pool(name="pocp", bufs=2) as pocp,
                    tc.tile_pool(name="pso", bufs=1, space="PSUM") as pso,
                    tc.tile_pool(name="psm", bufs=1, space="PSUM") as psm,
                ):
                    xt = stA.tile([P, NET, S], F8, tag="xt")
                    yt = stA.tile([P, NET, S], F8, tag="yt")
                    wxq = stA.tile([P, NET, D], F8, tag="wxq")
                    wyq = stA.tile([P, NET, D], F8, tag="wyq")
                    wfk = stA.tile([P, NCT, D], F8, tag="wfk")
                    wfv = stA.tile([P, NCT, D], F8, tag="wfv")
                    # DMA emission order = SP-queue order: minimal deps of the
                    # first matmul group (et=0: weight cols 0:128 of dt 0..1 +
                    # the ss=0 moving slices) first, then by first-use time.
                    # wfk/wfv ride the Activation HWDGE queue instead.
                    for dt in range(2):
                        nc.sync.dma_start(
                            wxq[:, dt, 0:P], wxq_d[dt, :, 0:P]
                        )
                    for dt in range(2):
                        nc.sync.dma_start(
                            xt[:, dt, 0:512], xt_d[dt, :, 0:512]
                        )
                    for dt in range(2):
                        nc.sync.dma_start(
                            wxq[:, dt, P:D], wxq_d[dt, :, P:D]
                        )
                    for dt in range(2, NET):
                        nc.sync.dma_start(wxq[:, dt], wxq_d[dt])
                    for dt in range(2, NET):
                        nc.sync.dma_start(
                            xt[:, dt, 0:512], xt_d[dt, :, 0:512]
                        )
                    for dt in range(NET):
                        nc.sync.dma_start(wyq[:, dt], wyq_d[dt])
                    for dt in range(NET):
                        nc.sync.dma_start(
                            yt[:, dt, 0:512], yt_d[dt, :, 0:512]
                        )
                    for ct in range(NCT):
                        nc.scalar.dma_start(wfk[:, ct], wfk_d[ct])
                    for ct in range(NCT):
                        nc.scalar.dma_start(wfv[:, ct], wfv_d[ct])
                    # Remaining 3/4 of X^T / Y^T as one large DMA per tile row.
                    for dt in range(NET):
                        nc.sync.dma_start(
                            xt[:, dt, 512:S], xt_d[dt, :, 512:S]
                        )
                    for dt in range(NET):
                        nc.sync.dma_start(
                            yt[:, dt, 512:S], yt_d[dt, :, 512:S]
                        )
                    # Residual inputs (first needed ~mid-projection).
                    for kt in range(NQT):
                        tx = work.tile([P, D], FP, tag="tx", name="tx")
                        ty = work.tile([P, D], FP, tag="ty", name="ty")
                        nc.sync.dma_start(tx[:], x_d[kt])
                        nc.sync.dma_start(ty[:], y_d[kt])
                        nc.gpsimd.tensor_add(racc[:, kt], tx[:], ty[:])

                    def proj_slice(ss):
                        """Q1/Q2, K^T, V for one 512-token slice.  Epilogue =
                        psum*1/WS + bias, cast to fp8; spread across scalar
                        (per-partition bias activation) and DVE."""
                        sl = slice(ss * 512, (ss + 1) * 512)
                        for si, (src, w, qdst, bcol) in enumerate((
                            (xt, wxq, q1t, 0),
                            (yt, wyq, q2t, 4),
                        )):
                            for et in range(NET):
                                ps = psS.tile([P, 512], FP, tag="psS", name="psS")
                                for d2 in range(NE2):
                                    nc.tensor.matmul(
                                        ps[:],
                                        (w[:, 2 * d2 : 2 * d2 + 2, et * P : (et + 1) * P]),
                                        (src[:, 2 * d2 : 2 * d2 + 2, sl]),
                                        start=d2 == 0,
                                        stop=d2 == NE2 - 1,
                                        perf_mode=DR,
                                    )
                                if (si + et) % 2 == 0:
                                    nc.scalar.activation(
                                        qdst[:, et, sl], ps[:], Ident,
                                        bias=bq[:, bcol + et : bcol + et + 1],
                                        scale=IWS,
                                    )
                                else:
                                    nc.vector.tensor_scalar(
                                        qdst[:, et, sl], ps[:], IWS,
                                        bq[:, bcol + et : bcol + et + 1],
                                        mult, add,
                                    )
                        for et in range(NET):
                            ps = psS.tile([P, 512], FP, tag="psS", name="psS")
                            for c2 in range(NC2):
                                qc = q1t if c2 < NE2 else q2t
                                co = (2 * c2) % NET
                                nc.tensor.matmul(
                                    ps[:],
                                    (wfk[:, 2 * c2 : 2 * c2 + 2, et * P : (et + 1) * P]),
                                    (qc[:, co : co + 2, sl]),
                                    start=c2 == 0,
                                    stop=c2 == NC2 - 1,
                                    perf_mode=DR,
                                )
                            nc.scalar.activation(
                                kft[:, et, sl], ps[:], Ident,
                                bias=bq[:, 8 + et : 9 + et],
                                scale=IWS,
                            )
                        for kt in range(4 * ss, 4 * ss + 4):
                            ps = psS.tile([P, D], FP, tag="psS", name="psS")
                            for c2 in range(NC2):
                                qc = q1t if c2 < NE2 else q2t
                                co = (2 * c2) % NET
                                nc.tensor.matmul(
                                    ps[:],
                                    (qc[:, co : co + 2, kt * P : (kt + 1) * P]),
                                    (wfv[:, 2 * c2 : 2 * c2 + 2]),
                                    start=c2 == 0,
                                    stop=c2 == NC2 - 1,
                                    perf_mode=DR,
                                )
                            nc.vector.scalar_tensor_tensor(
                                vf[:, kt], ps[:], IWS, bfv[:], op0=mult, op1=add
                            )

                    # ---- Attention (shared K/V, fp8 DoubleRow) ----
                    # PSUM: 4 O accumulators + 3 shared score banks + 1
                    # denominator = 8.  Denominator: ones8^T (x) es2 DoubleRow
                    # matmul per key chunk accumulates lane sums into
                    # pd [2, QB]; 4 PE transposes flip it to per-partition
                    # orientation for the reciprocal.  DVE evacuates/
                    # normalizes po, GPSIMD accumulates into racc.
                    def att_open(qb):
                        return {
                            "po": [
                                pso.tile([P, D], FP, name=f"po{qs}", tag=f"po{qs}")
                                for qs in range(NQS)
                            ],
                            "poc": [
                                pocp.tile([P, D], FP, name=f"poc{qs}", tag=f"poc{qs}")
                                for qs in range(NQS)
                            ],
                            "pd": psm.tile([2, QB], FP, tag="pd", name="pd"),
                        }

                    def att_k2(st, qsrc, qb, k2):
                        es2 = esp.tile([P, 2, QB], F8, tag="es2", name="es2")
                        for i in range(2):
                            kt = 2 * k2 + i
                            ps_s = psS.tile([P, QB], FP, tag="psS", name="psS")
                            for e2 in range(NE2):
                                nc.tensor.matmul(
                                    ps_s[:],
                                    (kft[:, 2 * e2 : 2 * e2 + 2, kt * P : (kt + 1) * P]),
                                    (qsrc[:, 2 * e2 : 2 * e2 + 2, qb * QB : (qb + 1) * QB]),
                                    start=e2 == 0,
                                    stop=e2 == NE2 - 1,
                                    perf_mode=DR,
                                )
                            nc.scalar.activation(
                                es2[:, i], ps_s[:], Exp, scale=ATT_SCALE
                            )
                        nc.tensor.matmul(
                            st["pd"][:],
                            (ones8[:, :, 0:2]),
                            (es2[:]),
                            start=k2 == 0,
                            stop=k2 == NK2 - 1,
                            perf_mode=DR,
                        )
                        for qs in range(NQS):
                            nc.tensor.matmul(
                                st["po"][qs][:],
                                (es2[:, :, qs * P : (qs + 1) * P]),
                                (vf[:, 2 * k2 : 2 * k2 + 2]),
                                start=k2 == 0,
                                stop=k2 == NK2 - 1,
                                perf_mode=DR,
                            )

                    def att_close(st, qi, qb, last_blk):
                        pdc = rcp.tile([2, QB], FP, tag="pdc", name="pdc")
                        nc.vector.tensor_copy(pdc[:], st["pd"][:])
                        # Transpose the [2, 512] lane-sum strip to per-
                        # partition orientation with 4 regular fp32 matmuls
                        # against one-hot selectors: pt[p, 2qs] = pdc[0,
                        # qs*128+p].  Each matmul writes the FULL [128, 8]
                        # region (zeros elsewhere), so the 4 of them form one
                        # well-defined PSUM accumulation group.
                        pt = psm.tile([P, 2 * NQS], FP, tag="pd", name="pt")
                        for qs in range(NQS):
                            nc.tensor.matmul(
                                pt[:],
                                (pdc[0:2, qs * P : (qs + 1) * P]),
                                (sel[:, qs]),
                                start=qs == 0,
                                stop=qs == NQS - 1,
                            )
                        rec = rcp.tile([P, 2 * NQS], FP, tag="rec", name="rec")
                        nc.vector.reciprocal(rec[:], pt[:])
                        for qs in range(NQS):
                            qt_i = qb * NQS + qs
                            if last_blk and qs % 2 == 0:
                                # Tail: split the last combines between DVE
                                # (straight from PSUM) and GPSIMD.
                                nc.vector.scalar_tensor_tensor(
                                    racc[:, qt_i],
                                    st["po"][qs][:],
                                    rec[:, 2 * qs : 2 * qs + 1],
                                    racc[:, qt_i],
                                    op0=mult,
                                    op1=add,
                                )
                            else:
                                # Normalize on DVE (evacuates the po bank),
                                # accumulate into racc on GPSIMD (which
                                # supports only plain TensorTensor ops).
                                nc.vector.tensor_scalar_mul(
                                    st["poc"][qs][:],
                                    st["po"][qs][:],
                                    rec[:, 2 * qs : 2 * qs + 1],
                                )
                                nc.gpsimd.tensor_add(
                                    racc[:, qt_i], racc[:, qt_i],
                                    st["poc"][qs][:],
                                )
                            if qi == 1:
                                # racc final for this q-subtile: start the
                                # output DMA so it overlaps the rest of the
                                # second attention pass.
                                nc.sync.dma_start(out_d[qt_i], racc[:, qt_i])

                    # Interleaved prologue: two projection slices, then the
                    # first attention block with the remaining slices emitted
                    # between its key-chunks (keys kt arrive slice by slice).
                    proj_slice(0)
                    proj_slice(1)
                    st0 = att_open(0)
                    for k2 in range(4):
                        att_k2(st0, q1t, 0, k2)
                    proj_slice(2)
                    for k2 in range(4, 6):
                        att_k2(st0, q1t, 0, k2)
                    proj_slice(3)
                    for k2 in range(6, NK2):
                        att_k2(st0, q1t, 0, k2)
                    att_close(st0, 0, 0, False)
                    for qi, qsrc in enumerate((q1t, q2t)):
                        for qb in range(NQB):
                            if qi == 0 and qb == 0:
                                continue
                            st = att_open(qb)
                            for k2 in range(NK2):
                                att_k2(st, qsrc, qb, k2)
                            att_close(st, qi, qb, qi == 1 and qb == NQB - 1)

    nc.compile()
    return nc


def get_nc(reps: int = 1):
    if reps not in _CACHE:
        _CACHE[reps] = _build(reps)
    return _CACHE[reps]


def make_in_maps(X, Y, W_xq, b_xq, W_yq, b_yq, W_fk, b_fk, W_fv, b_fv):
    """Host-side layout prep (transposes / fp8 quantization; weights
    pre-scaled by WS) and per-core sharding over batch."""
    f32 = np.float32

    def c(a):
        return np.ascontiguousarray(a, dtype=f32)

    def q8(a):
        return np.ascontiguousarray(
            np.asarray(a, dtype=f32), dtype=ml_dtypes.float8_e4m3
        )

    wxq = q8(W_xq.T * WS).reshape(NET, P, D)
    wyq = q8(W_yq.T * WS).reshape(NET, P, D)
    wfk = q8(W_fk.T * WS).reshape(NCT, P, D)
    wfv = q8(W_fv.T * WS).reshape(NCT, P, D)
    bq = np.empty((P, 12), f32)
    bq[:, 0:4] = b_xq.reshape(NET, P).T
    bq[:, 4:8] = b_yq.reshape(NET, P).T
    bq[:, 8:12] = b_fk.reshape(NET, P).T
    bfv = c(np.broadcast_to(b_fv.astype(f32), (P, D)))
    sel = np.zeros((2, NQS, 2 * NQS), f32)
    for qs in range(NQS):
        sel[0, qs, 2 * qs] = 1.0
        sel[0, qs, 2 * qs + 1] = 1.0

    in_maps = []
    for b in range(X.shape[0]):
        in_maps.append(
            {
                "xt": q8(X[b].T).reshape(NET, P, S),
                "yt": q8(Y[b].T).reshape(NET, P, S),
                "x": c(X[b].reshape(NQT, P, D)),
                "y": c(Y[b].reshape(NQT, P, D)),
                "wxq": wxq,
                "wyq": wyq,
                "wfk": wfk,
                "wfv": wfv,
                "bq": bq,
                "bfv": bfv,
                "sel": sel,
            }
        )
    return in_maps


def kernel(X, Y, W_xq, b_xq, W_yq, b_yq, W_fk, b_fk, W_fv, b_fv):
    X = np.asarray(X, np.float32)
    Y = np.asarray(Y, np.float32)
    B = X.shape[0]
    nc = get_nc()
    in_maps = make_in_maps(
        X, Y,
        np.asarray(W_xq, np.float32), np.asarray(b_xq, np.float32),
        np.asarray(W_yq, np.float32), np.asarray(b_yq, np.float32),
        np.asarray(W_fk, np.float32), np.asarray(b_fk, np.float32),
        np.asarray(W_fv, np.float32), np.asarray(b_fv, np.float32),
    )
    res = run_bass_kernel_spmd(nc, in_maps, list(range(B)))
    out = np.stack([res.results[b]["out"].reshape(S, D) for b in range(B)])
    return out


# revision 30
# speedup vs baseline: 1.0702x; 1.0702x over previous
"""Trainium2 Bass kernel for the CA2 dense-transformer problem.

Math (per batch b of 8, S=2048, D=512):
    Q1 = X @ W_xq.T + b_xq            # [S, D]
    Q2 = Y @ W_yq.T + b_yq
    Qc = concat(Q1, Q2, -1)           # [S, 2D]
    K  = Qc @ W_fk.T + b_fk
    V  = Qc @ W_fv.T + b_fv
    out = X + Y + softmax(Q1 K^T / sqrt(D)) V + softmax(Q2 K^T / sqrt(D)) V

Sharding: pure data-parallel over batch; core i handles batch i.

Numerics: every matmul runs in fp8e4 (e4m3) with DoubleRow perf mode
(measured ~0.5 PE cycles per output column on TRN2, 4x the fp32r rate),
accumulating in fp32 PSUM.  Weights are pre-scaled by 2^12 on the host so
their small uniform(-0.03..0.04) entries land in e4m3's normal range; the
2^-12 descale is folded into the fp32 epilogue.  The attention 1/sqrt(D)
scale is folded into the Exp activation's scale operand.  The softmax
denominator, residual X+Y, and output all stay fp32.  The attention
contribution is ~4% of the output norm, so fp8's ~2-3% elementwise error
dilutes to <1e-3 relative error.

Schedule: engines execute their queues in emission order.  The
projections are emitted pipelined per 512-token slice (Qx, Qy, K, V per
slice), and the first attention block (pass 1, q-block 0 — whose queries
only need slice 0) starts after just two slices, with the remaining two
slices interleaved between its key-chunks, sharing one 3-buffer PSUM
pool between projection psums and score psums (3 + 4 po + 1 pd = 8
banks).  Non-PE work is spread: scalar does exp + the K and half the Q
epilogues, DVE the rest + PSUM evacuations, GPSIMD residual init and the
final accumulate into racc.  The softmax denominator runs on the tensor
engine: a DoubleRow ones-matmul per key-chunk accumulates lane sums into
a [2, 512] PSUM strip, transposed to per-partition orientation by 4 tiny
PE transposes.
"""

import sys

if "/opt/trn_rl_repo" not in sys.path:
    sys.path.insert(0, "/opt/trn_rl_repo")

import ml_dtypes
import numpy as np

import concourse.bass as bass  # noqa: F401  (bass types used via tile/bacc)
import concourse.mybir as mybir
import concourse.tile as tile
from concourse import bacc
from concourse.bass_utils import run_bass_kernel_spmd

P = 128          # SBUF partitions
S = 2048         # tokens per batch
D = 512          # feature dim
NQT = S // P     # 16 token tiles
NET = D // P     # 4 feature tiles of D
NCT = 2 * D // P # 8 feature tiles of 2D
NE2 = NET // 2   # 2 double (256-deep) feature tiles of D
NC2 = NCT // 2   # 4 double feature tiles of 2D
NK2 = NQT // 2   # 8 double key tiles
NSS = S // 512   # 4 512-wide token column slices
QB = 512         # q-block columns processed together in attention
NQB = S // QB    # 4
NQS = QB // P    # 4 q-subtiles per block
FP = mybir.dt.float32
F8 = mybir.dt.float8e4
DR = mybir.MatmulPerfMode.DoubleRow
WS = 2.0 ** 12   # host-side weight pre-scale (max |w|*WS ~ 181 < 240)
IWS = 1.0 / WS

_CACHE = {}


def _build(reps: int = 1):
    nc = bacc.Bacc("TRN2", target_bir_lowering=False, debug=False)

    xt_d = nc.dram_tensor("xt", [NET, P, S], F8, kind="ExternalInput")
    yt_d = nc.dram_tensor("yt", [NET, P, S], F8, kind="ExternalInput")
    x_d = nc.dram_tensor("x", [NQT, P, D], FP, kind="ExternalInput")
    y_d = nc.dram_tensor("y", [NQT, P, D], FP, kind="ExternalInput")
    wxq_d = nc.dram_tensor("wxq", [NET, P, D], F8, kind="ExternalInput")
    wyq_d = nc.dram_tensor("wyq", [NET, P, D], F8, kind="ExternalInput")
    wfk_d = nc.dram_tensor("wfk", [NCT, P, D], F8, kind="ExternalInput")
    wfv_d = nc.dram_tensor("wfv", [NCT, P, D], F8, kind="ExternalInput")
    bq_d = nc.dram_tensor("bq", [P, 12], FP, kind="ExternalInput")
    bfv_d = nc.dram_tensor("bfv", [P, D], FP, kind="ExternalInput")
    sel_d = nc.dram_tensor("sel", [2, NQS, 2 * NQS], mybir.dt.bfloat16,
                           kind="ExternalInput")
    out_d = nc.dram_tensor("out", [NQT, P, D], FP, kind="ExternalOutput")

    Exp = mybir.ActivationFunctionType.Exp
    Ident = mybir.ActivationFunctionType.Identity
    mult = mybir.AluOpType.mult
    add = mybir.AluOpType.add
    ATT_SCALE = float(1.0 / np.sqrt(np.float32(D)))

    with tile.TileContext(nc) as tc:
        for _rep in range(reps):
            with (
                tc.tile_pool(name="main", bufs=1) as main,
                tc.tile_pool(name="work", bufs=3) as work,
            ):
                q1t = main.tile([P, NET, S], F8, tag="q1t")
                q2t = main.tile([P, NET, S], F8, tag="q2t")
                kft = main.tile([P, NET, S], F8, tag="kft")
                vf = main.tile([P, NQT, D], F8, tag="vf")
                racc = main.tile([P, NQT, D], FP, tag="racc")
                bq = main.tile([P, 12], FP, tag="bq")
                bfv = main.tile([P, D], FP, tag="bfv")
                # DoubleRow ldweights requires the k-pair dim stride to be a
                # multiple of 16 elements, so pad the ones tile to [P, 2, 16].
                ones8 = main.tile([P, 2, 16], F8, tag="ones8")
                sel = main.tile([2, NQS, 2 * NQS], mybir.dt.bfloat16, tag="sel")
                nc.scalar.dma_start(bq[:], bq_d[:])
                nc.scalar.dma_start(bfv[:], bfv_d[:])
                nc.scalar.dma_start(sel[:], sel_d[:])
                nc.vector.memset(ones8[:], 1.0)

                with (
                    tc.tile_pool(name="stA", bufs=1) as stA,
                    tc.tile_pool(name="psS", bufs=3, space="PSUM") as psS,
                    tc.tile_pool(name="esp", bufs=3) as esp,
                    tc.tile_pool(name="rcp", bufs=2) as rcp,
                    tc.tile_pool(name="pocp", bufs=2) as pocp,
                    tc.tile_pool(name="pso", bufs=1, space="PSUM") as pso,
                    tc.tile_pool(name="psm", bufs=1, space="PSUM") as psm,
                ):
                    xt = stA.tile([P, NET, S], F8, tag="xt")
                    yt = stA.tile([P, NET, S], F8, tag="yt")
                    wxq = stA.tile([P, NET, D], F8, tag="wxq")
                    wyq = stA.tile([P, NET, D], F8, tag="wyq")
                    wfk = stA.tile([P, NCT, D], F8, tag="wfk")
                    wfv = stA.tile([P, NCT, D], F8, tag="wfv")
                    # DMA emission order = SP-queue order: minimal deps of the
                    # first matmul group (et=0: weight cols 0:128 of dt 0..1 +
                    # the ss=0 moving slices) first, then by first-use time.
                    # wfk/wfv ride the Activation HWDGE queue instead.
                    for dt in range(2):
                        nc.sync.dma_start(
                            wxq[:, dt, 0:P], wxq_d[dt, :, 0:P]
                        )
                    for dt in range(2):
                        nc.sync.dma_start(
                            xt[:, dt, 0:512], xt_d[dt, :, 0:512]
                        )
                    for dt in range(2):
                        nc.sync.dma_start(
                            wxq[:, dt, P:D], wxq_d[dt, :, P:D]
                        )
                    for dt in range(2, NET):
                        nc.sync.dma_start(wxq[:, dt], wxq_d[dt])
                    for dt in range(2, NET):
                        nc.sync.dma_start(
                            xt[:, dt, 0:512], xt_d[dt, :, 0:512]
                        )
                    for dt in range(NET):
                        nc.sync.dma_start(wyq[:, dt], wyq_d[dt])
                    for dt in range(NET):
                        nc.sync.dma_start(
                            yt[:, dt, 0:512], yt_d[dt, :, 0:512]
                        )
                    for ct in range(NCT):
                        nc.scalar.dma_start(wfk[:, ct], wfk_d[ct])
                    for ct in range(NCT):
                        nc.scalar.dma_start(wfv[:, ct], wfv_d[ct])
                    # Remaining 3/4 of X^T / Y^T as one large DMA per tile row.
                    for dt in range(NET):
                        nc.sync.dma_start(
                            xt[:, dt, 512:S], xt_d[dt, :, 512:S]
                        )
                    for dt in range(NET):
                        nc.sync.dma_start(
                            yt[:, dt, 512:S], yt_d[dt, :, 512:S]
                        )
                    # Residual inputs (first needed ~mid-projection).
                    for kt in range(NQT):
                        tx = work.tile([P, D], FP, tag="tx", name="tx")
                        ty = work.tile([P, D], FP, tag="ty", name="ty")
                        nc.sync.dma_start(tx[:], x_d[kt])
                        nc.sync.dma_start(ty[:], y_d[kt])
                        nc.gpsimd.tensor_add(racc[:, kt], tx[:], ty[:])

                    def proj_slice(ss):
                        """Q1/Q2, K^T, V for one 512-token slice.  Epilogue =
                        psum*1/WS + bias, cast to fp8; spread across scalar
                        (per-partition bias activation) and DVE."""
                        sl = slice(ss * 512, (ss + 1) * 512)
                        for si, (src, w, qdst, bcol) in enumerate((
                            (xt, wxq, q1t, 0),
                            (yt, wyq, q2t, 4),
                        )):
                            for et in range(NET):
                                ps = psS.tile([P, 512], FP, tag="psS", name="psS")
                                for d2 in range(NE2):
                                    nc.tensor.matmul(
                                        ps[:],
                                        (w[:, 2 * d2 : 2 * d2 + 2, et * P : (et + 1) * P]),
                                        (src[:, 2 * d2 : 2 * d2 + 2, sl]),
                                        start=d2 == 0,
                                        stop=d2 == NE2 - 1,
                                        perf_mode=DR,
                                    )
                                if (si + et) % 2 == 0:
                                    nc.scalar.activation(
                                        qdst[:, et, sl], ps[:], Ident,
                                        bias=bq[:, bcol + et : bcol + et + 1],
                                        scale=IWS,
                                    )
                                else:
                                    nc.vector.tensor_scalar(
                                        qdst[:, et, sl], ps[:], IWS,
                                        bq[:, bcol + et : bcol + et + 1],
                                        mult, add,
                                    )
                        for et in range(NET):
                            ps = psS.tile([P, 512], FP, tag="psS", name="psS")
                            for c2 in range(NC2):
                                qc = q1t if c2 < NE2 else q2t
                                co = (2 * c2) % NET
                                nc.tensor.matmul(
                                    ps[:],
                                    (wfk[:, 2 * c2 : 2 * c2 + 2, et * P : (et + 1) * P]),
                                    (qc[:, co : co + 2, sl]),
                                    start=c2 == 0,
                                    stop=c2 == NC2 - 1,
                                    perf_mode=DR,
                                )
                            nc.scalar.activation(
                                kft[:, et, sl], ps[:], Ident,
                                bias=bq[:, 8 + et : 9 + et],
                                scale=IWS,
                            )
                        for kt in range(4 * ss, 4 * ss + 4):
                            ps = psS.tile([P, D], FP, tag="psS", name="psS")
                            for c2 in range(NC2):
                                qc = q1t if c2 < NE2 else q2t
                                co = (2 * c2) % NET
                                nc.tensor.matmul(
                                    ps[:],
                                    (qc[:, co : co + 2, kt * P : (kt + 1) * P]),
                                    (wfv[:, 2 * c2 : 2 * c2 + 2]),
                                    start=c2 == 0,
                                    stop=c2 == NC2 - 1,
                                    perf_mode=DR,
                                )
                            nc.vector.scalar_tensor_tensor(
                                vf[:, kt], ps[:], IWS, bfv[:], op0=mult, op1=add
                            )

                    # ---- Attention (shared K/V, fp8 DoubleRow) ----
                    # PSUM: 4 O accumulators + 3 shared score banks + 1
                    # denominator = 8.  Denominator: ones8^T (x) es2 DoubleRow
                    # matmul per key chunk accumulates lane sums into
                    # pd [2, QB]; 4 PE transposes flip it to per-partition
                    # orientation for the reciprocal.  DVE evacuates/
                    # normalizes po, GPSIMD accumulates into racc.
                    def att_open(qb):
                        return {
                            "po": [
                                pso.tile([P, D], FP, name=f"po{qs}", tag=f"po{qs}")
                                for qs in range(NQS)
                            ],
                            "poc": [
                                pocp.tile([P, D], FP, name=f"poc{qs}", tag=f"poc{qs}")
                                for qs in range(NQS)
                            ],
                            "pd": psm.tile([2, QB], FP, tag="pd", name="pd"),
                        }

                    def att_k2(st, qsrc, qb, k2):
                        es2 = esp.tile([P, 2, QB], F8, tag="es2", name="es2")
                        for i in range(2):
                            kt = 2 * k2 + i
                            ps_s = psS.tile([P, QB], FP, tag="psS", name="psS")
                            for e2 in range(NE2):
                                nc.tensor.matmul(
                                    ps_s[:],
                                    (kft[:, 2 * e2 : 2 * e2 + 2, kt * P : (kt + 1) * P]),
                                    (qsrc[:, 2 * e2 : 2 * e2 + 2, qb * QB : (qb + 1) * QB]),
                                    start=e2 == 0,
                                    stop=e2 == NE2 - 1,
                                    perf_mode=DR,
                                )
                            nc.scalar.activation(
                                es2[:, i], ps_s[:], Exp, scale=ATT_SCALE
                            )
                        nc.tensor.matmul(
                            st["pd"][:],
                            (ones8[:, :, 0:2]),
                            (es2[:]),
                            start=k2 == 0,
                            stop=k2 == NK2 - 1,
                            perf_mode=DR,
                        )
                        for qs in range(NQS):
                            nc.tensor.matmul(
                                st["po"][qs][:],
                                (es2[:, :, qs * P : (qs + 1) * P]),
                                (vf[:, 2 * k2 : 2 * k2 + 2]),
                                start=k2 == 0,
                                stop=k2 == NK2 - 1,
                                perf_mode=DR,
                            )

                    def att_close(st, qi, qb, last_blk):
                        # bf16: exact for the 0/1 selectors; ~0.4% on the
                        # denominators, diluted ~35x in the output.  fp32
                        # matmuls here would cost ~1.3us each (self-loading
                        # weights); bf16 is ~0.1us.
                        pdc = rcp.tile([2, QB], mybir.dt.bfloat16,
                                       tag="pdc", name="pdc")
                        nc.vector.tensor_copy(pdc[:], st["pd"][:])
                        # Transpose the [2, 512] lane-sum strip to per-
                        # partition orientation with 4 regular fp32 matmuls
                        # against one-hot selectors: pt[p, 2qs] = pdc[0,
                        # qs*128+p].  Each matmul writes the FULL [128, 8]
                        # region (zeros elsewhere), so the 4 of them form one
                        # well-defined PSUM accumulation group.
                        pt = psm.tile([P, 2 * NQS], FP, tag="pd", name="pt")
                        for qs in range(NQS):
                            nc.tensor.matmul(
                                pt[:],
                                (pdc[0:2, qs * P : (qs + 1) * P]),
                                (sel[:, qs]),
                                start=qs == 0,
                                stop=qs == NQS - 1,
                            )
                        rec = rcp.tile([P, 2 * NQS], FP, tag="rec", name="rec")
                        nc.vector.reciprocal(rec[:], pt[:])
                        for qs in range(NQS):
                            qt_i = qb * NQS + qs
                            if last_blk and qs % 2 == 0:
                                # Tail: split the last combines between DVE
                                # (straight from PSUM) and GPSIMD.
                                nc.vector.scalar_tensor_tensor(
                                    racc[:, qt_i],
                                    st["po"][qs][:],
                                    rec[:, 2 * qs : 2 * qs + 1],
                                    racc[:, qt_i],
                                    op0=mult,
                                    op1=add,
                                )
                            else:
                                # Normalize on DVE (evacuates the po bank),
                                # accumulate into racc on GPSIMD (which
                                # supports only plain TensorTensor ops).
                                nc.vector.tensor_scalar_mul(
                                    st["poc"][qs][:],
                                    st["po"][qs][:],
                                    rec[:, 2 * qs : 2 * qs + 1],
                                )
                                nc.gpsimd.tensor_add(
                                    racc[:, qt_i], racc[:, qt_i],
                                    st["poc"][qs][:],
                                )
                            if qi == 1:
                                # racc final for this q-subtile: start the
                                # output DMA so it overlaps the rest of the
                                # second attention pass.
                                nc.sync.dma_start(out_d[qt_i], racc[:, qt_i])

                    # Interleaved prologue: two projection slices, then the
                    # first attention block with the remaining slices emitted
                    # between its key-chunks (keys kt arrive slice by slice).
                    proj_slice(0)
                    proj_slice(1)
                    st0 = att_open(0)
                    for k2 in range(4):
                        att_k2(st0, q1t, 0, k2)
                    proj_slice(2)
                    for k2 in range(4, 6):
                        att_k2(st0, q1t, 0, k2)
                    proj_slice(3)
                    for k2 in range(6, NK2):
                        att_k2(st0, q1t, 0, k2)
                    att_close(st0, 0, 0, False)
                    for qi, qsrc in enumerate((q1t, q2t)):
                        for qb in range(NQB):
                            if qi == 0 and qb == 0:
                                continue
                            st = att_open(qb)
                            for k2 in range(NK2):
                                att_k2(st, qsrc, qb, k2)
                            att_close(st, qi, qb, qi == 1 and qb == NQB - 1)

    nc.compile()
    return nc


def get_nc(reps: int = 1):
    if reps not in _CACHE:
        _CACHE[reps] = _build(reps)
    return _CACHE[reps]


def make_in_maps(X, Y, W_xq, b_xq, W_yq, b_yq, W_fk, b_fk, W_fv, b_fv):
    """Host-side layout prep (transposes / fp8 quantization; weights
    pre-scaled by WS) and per-core sharding over batch."""
    f32 = np.float32

    def c(a):
        return np.ascontiguousarray(a, dtype=f32)

    def q8(a):
        return np.ascontiguousarray(
            np.asarray(a, dtype=f32), dtype=ml_dtypes.float8_e4m3
        )

    wxq = q8(W_xq.T * WS).reshape(NET, P, D)
    wyq = q8(W_yq.T * WS).reshape(NET, P, D)
    wfk = q8(W_fk.T * WS).reshape(NCT, P, D)
    wfv = q8(W_fv.T * WS).reshape(NCT, P, D)
    bq = np.empty((P, 12), f32)
    bq[:, 0:4] = b_xq.reshape(NET, P).T
    bq[:, 4:8] = b_yq.reshape(NET, P).T
    bq[:, 8:12] = b_fk.reshape(NET, P).T
    bfv = c(np.broadcast_to(b_fv.astype(f32), (P, D)))
    sel = np.zeros((2, NQS, 2 * NQS), ml_dtypes.bfloat16)
    for qs in range(NQS):
        sel[0, qs, 2 * qs] = 1.0
        sel[0, qs, 2 * qs + 1] = 1.0

    in_maps = []
    for b in range(X.shape[0]):
        in_maps.append(
            {
                "xt": q8(X[b].T).reshape(NET, P, S),
                "yt": q8(Y[b].T).reshape(NET, P, S),
                "x": c(X[b].reshape(NQT, P, D)),
                "y": c(Y[b].reshape(NQT, P, D)),
                "wxq": wxq,
                "wyq": wyq,
                "wfk": wfk,
                "wfv": wfv,
                "bq": bq,
                "bfv": bfv,
                "sel": sel,
            }
        )
    return in_maps


def kernel(X, Y, W_xq, b_xq, W_yq, b_yq, W_fk, b_fk, W_fv, b_fv):
    X = np.asarray(X, np.float32)
    Y = np.asarray(Y, np.float32)
    B = X.shape[0]
    nc = get_nc()
    in_maps = make_in_maps(
        X, Y,
        np.asarray(W_xq, np.float32), np.asarray(b_xq, np.float32),
        np.asarray(W_yq, np.float32), np.asarray(b_yq, np.float32),
        np.asarray(W_fk, np.float32), np.asarray(b_fk, np.float32),
        np.asarray(W_fv, np.float32), np.asarray(b_fv, np.float32),
    )
    res = run_bass_kernel_spmd(nc, in_maps, list(range(B)))
    out = np.stack([res.results[b]["out"].reshape(S, D) for b in range(B)])
    return out
